# revision 27
# baseline (speedup 1.0000x reference)
"""Batched attention (no-scale softmax) for Trainium2, 8 NeuronCores.

Problem: q [16,2048,128] f32, k [16,128,2048] f32, v [16,2048,128] f32
         out = softmax(q @ k, axis=-1) @ v          -> [16,2048,128] f32

Sharding: batch dim split across 8 cores (2 batches/core), no communication.

Per-core design:
  - scores^T [j, i] straight from the PE: lhsT = k-block [d, j] (fp32r),
    rhs = q^T [d, i] (fp32r; q transposed once per batch on the PE).
  - exp on ACT (PSUM -> SBUF, bf16), 1024-wide calls. No max subtraction:
    scores ~ N(0,128) so |s| < ~70 and exp stays in fp32/bf16 range;
    softmax is shift-invariant so this matches the reference up to fp error.
  - PV: lhsT = exp^T block [j, i-block] (stationary, bf16), rhs = v-block
    augmented with a ones column [j, 128+1] -> accumulates [i, d | sum] in
    PSUM over j. Column 128 is the softmax denominator (free), and the
    output lands directly in [i, d] layout on the right partitions.
  - normalize: out = acc[:, :128] * (1 / acc[:, 128]) per partition, DMA out.
"""

import sys

sys.path.insert(0, "/opt/trn_rl_repo")

import numpy as np

import concourse.bacc as bacc
import concourse.tile as tile
from concourse import mybir
from concourse.bass_utils import run_bass_kernel_spmd
from concourse.masks import make_identity

B, N, D = 16, 2048, 128
N_CORES = 8
BPC = B // N_CORES  # batches per core
NT = N // 128  # 16 blocks of 128 along N
IW = 1024  # i-width per QK/exp pass (ACT call width)
NIH = N // IW  # 2
QW = 512  # i-width per PV pass (4 PSUM-bank accumulators)

F32 = mybir.dt.float32
F32R = mybir.dt.float32r
BF16 = mybir.dt.bfloat16


def build_nc(qk_f32r=True, probs_bf16=True, repeat=1, st_bufs=2, et_bufs=36):
    nc = bacc.Bacc(
        "TRN2", target_bir_lowering=False, debug=False, enable_asserts=False
    )
    q_d = nc.dram_tensor("q", [BPC, N, D], F32, kind="ExternalInput").ap()
    k_d = nc.dram_tensor("k", [BPC, D, N], F32, kind="ExternalInput").ap()
    v_d = nc.dram_tensor("v", [BPC, N, D], F32, kind="ExternalInput").ap()
    o_d = nc.dram_tensor("out", [BPC, N, D], F32, kind="ExternalOutput").ap()

    PDT = BF16 if probs_bf16 else F32
    QKDT = F32R if qk_f32r else F32

    with tile.TileContext(nc) as tc:
        with (
            tc.tile_pool(name="consts", bufs=1) as consts,
            tc.tile_pool(name="kp", bufs=2) as kp,
            tc.tile_pool(name="qp", bufs=2) as qp,
            tc.tile_pool(name="qtp", bufs=2) as qtp,
            tc.tile_pool(name="vfp", bufs=2) as vfp,
            tc.tile_pool(name="vbp", bufs=2) as vbp,
            tc.tile_pool(name="etp", bufs=et_bufs) as etp,
            tc.tile_pool(name="osp", bufs=4) as osp,
            tc.tile_pool(name="rsp", bufs=4) as rsp,
            tc.tile_pool(name="stp", bufs=st_bufs, space="PSUM") as stp,
            tc.tile_pool(name="oap", bufs=4, space="PSUM") as oap,
        ):
            identity = consts.tile([128, 128], F32)
            make_identity(nc, identity)

            def load_batch(b):
                """Input DMAs on the sync (HWDGE) queue. Few, large chunks
                (per-DMA queue overhead is significant), ordered so early
                compute dependencies (q for transposes, k block 0, v for
                the deferred PV) land first."""
                q_sb = qp.tile([128, NT, 128], F32, tag="q", name="q_sb")
                q_src = q_d[b].rearrange("(t p) d -> p t d", p=128)
                k_sb = kp.tile([128, N], QKDT, tag="k", name="k_sb")
                k_src = k_d[b].bitcast(QKDT)
                vf_sb = vfp.tile([128, NT, 128], F32, tag="vf", name="vf_sb")
                v_src = v_d[b].rearrange("(t p) d -> p t d", p=128)
                nc.sync.dma_start(out=q_sb[:, 0:4, :], in_=q_src[:, 0:4, :])
                nc.sync.dma_start(
                    out=k_sb[:, 0 : 2 * 128], in_=k_src[:, 0 : 2 * 128]
                )
                nc.sync.dma_start(out=q_sb[:, 4:8, :], in_=q_src[:, 4:8, :])
                nc.sync.dma_start(
                    out=k_sb[:, 2 * 128 : 5 * 128], in_=k_src[:, 2 * 128 : 5 * 128]
                )
                nc.sync.dma_start(out=vf_sb[:, 0:8, :], in_=v_src[:, 0:8, :])
                nc.sync.dma_start(out=q_sb[:, 8:NT, :], in_=q_src[:, 8:NT, :])
                nc.sync.dma_start(out=vf_sb[:, 8:NT, :], in_=v_src[:, 8:NT, :])
                nc.sync.dma_start(
                    out=k_sb[:, 5 * 128 : 10 * 128], in_=k_src[:, 5 * 128 : 10 * 128]
                )
                nc.sync.dma_start(
                    out=k_sb[:, 10 * 128 :], in_=k_src[:, 10 * 128 :]
                )
                return q_sb, k_sb, vf_sb

            NB = IW // 128  # 8 i-blocks per unit
            NCK = QW // 128  # 4 accumulators per PV pass

            def pv_chunk(p, iq, jc):
                """One j-chunk of the deferred PV pass `iq` for unit `p`."""
                if jc == 0:
                    p["oaccs"][iq] = [
                        oap.tile([128, 129], F32, tag="oa", name="oacc")
                        for _ in range(NCK)
                    ]
                for ib in range(NCK):
                    nc.tensor.matmul(
                        p["oaccs"][iq][ib],
                        lhsT=p["ets"][jc][
                            :, iq * QW + ib * 128 : iq * QW + (ib + 1) * 128
                        ],
                        rhs=p["v_aug"][:, jc, :],
                        start=(jc == 0),
                        stop=(jc == NT - 1),
                    )

            def pv_readout(p, iq):
                """Normalize + store the 4 blocks of pass `iq` of unit `p`."""
                for ib in range(NCK):
                    rs = rsp.tile([128, 1], F32, tag="rs", name="rs")
                    nc.vector.reciprocal(out=rs, in_=p["oaccs"][iq][ib][:, 128:129])
                    out_sb = osp.tile([128, 128], F32, tag="os", name="out_sb")
                    nc.vector.tensor_scalar_mul(
                        out_sb, p["oaccs"][iq][ib][:, 0:128], rs
                    )
                    t = p["ih"] * NB + iq * NCK + ib
                    nc.sync.dma_start(
                        out=o_d[p["b"], t * 128 : (t + 1) * 128, :], in_=out_sb
                    )

            iters = [b for _ in range(repeat) for b in range(BPC)]
            NU = len(iters) * NIH
            loaded = [None] * len(iters)
            res = [None] * len(iters)

            def ensure_loaded(it):
                if loaded[it] is None:
                    loaded[it] = load_batch(iters[it])

            def ensure_res(it):
                if res[it] is None:
                    ensure_loaded(it)
                    q_sb, k_sb, vf_sb = loaded[it]
                    # v blocks with a ones column: [j, 0:128]=v, [j, 128]=1
                    # (bf16 conversion split in halves so the deferred PV can
                    # start on the first half as soon as it lands)
                    v_aug = vbp.tile([128, NT, 129], PDT, tag="vb", name="v_aug")
                    nc.vector.memset(v_aug[:, :, 128:129], 1.0)
                    nc.gpsimd.tensor_copy(
                        out=v_aug[:, 0:8, 0:128], in_=vf_sb[:, 0:8, :]
                    )
                    nc.gpsimd.tensor_copy(
                        out=v_aug[:, 8:NT, 0:128], in_=vf_sb[:, 8:NT, :]
                    )
                    qT_sb = qtp.tile([128, N], QKDT, tag="qt", name="qT_sb")
                    res[it] = {"q": q_sb, "k": k_sb, "v": v_aug, "qT": qT_sb}

            def emit_qt(u, ts):
                """PE-transpose q blocks `ts` of unit u into its qT buffer."""
                it, ih = divmod(u, NIH)
                ensure_res(it)
                r = res[it]
                for t in ts:
                    qt_ps = stp.tile([128, 128], F32, tag="st", name="qt_ps")
                    nc.tensor.transpose(qt_ps, r["q"][:, t, :], identity)
                    nc.vector.tensor_copy(
                        out=r["qT"][:, t * 128 : (t + 1) * 128], in_=qt_ps
                    )

            pending = None  # previous unit, PV deferred into the current unit
            emit_qt(0, range(NB))
            for u in range(NU):
                it, ih = divmod(u, NIH)
                b = iters[it]
                r = res[it]
                i0 = ih * IW
                if ih == 0 and it + 1 < len(iters):
                    # prefetch next iteration's inputs ahead in DMA order
                    ensure_loaded(it + 1)
                # QK + exp pipeline. Interleaved between QK steps: the
                # PREVIOUS unit's PV matmuls (jb 0..15) and the NEXT unit's
                # q^T transposes (jb 8..15) — ACT stays saturated and
                # neither PV nor q^T sits on the inter-unit critical path.
                ets = []
                for jb in range(NT):
                    st = stp.tile([128, IW], F32, tag="st", name="st")
                    for c in range(IW // 512):
                        nc.tensor.matmul(
                            st[:, c * 512 : (c + 1) * 512],
                            lhsT=r["k"][:, jb * 128 : (jb + 1) * 128],
                            rhs=r["qT"][:, i0 + c * 512 : i0 + (c + 1) * 512],
                            start=True,
                            stop=True,
                        )
                    et = etp.tile([128, IW], PDT, tag="et", name="et")
                    nc.scalar.activation(
                        out=et, in_=st, func=mybir.ActivationFunctionType.Exp
                    )
                    ets.append(et)
                    if pending is not None:
                        iq, jc0 = divmod(jb, NT // 2)
                        pv_chunk(pending, iq, 2 * jc0)
                        pv_chunk(pending, iq, 2 * jc0 + 1)
                        if jb == NT // 2 - 1:
                            pv_readout(pending, 0)
                        elif jb == NT - 1:
                            pv_readout(pending, 1)
                    if jb >= NT - NB and u + 1 < NU:
                        nih = (u + 1) % NIH
                        emit_qt(u + 1, [nih * NB + (jb - (NT - NB))])
                pending = {
                    "b": b,
                    "ih": ih,
                    "ets": ets,
                    "v_aug": r["v"],
                    "oaccs": [None, None],
                }

            # flush the last unit's PV
            for iq in range(IW // QW):
                for jc in range(NT):
                    pv_chunk(pending, iq, jc)
                pv_readout(pending, iq)

    nc.compile()
    return nc


_NC_CACHE = {}


def _get_nc(key=()):
    if key not in _NC_CACHE:
        _NC_CACHE[key] = build_nc(*key)
    return _NC_CACHE[key]


_RUNNER = None


def _get_runner():
    """Persistent jitted shard_map runner (one XLA wrapper + NEFF compile,
    reused across kernel() calls)."""
    global _RUNNER
    if _RUNNER is not None:
        return _RUNNER
    import jax
    from jax.sharding import Mesh, PartitionSpec, NamedSharding

    try:
        from jax import shard_map
    except ImportError:
        from jax.experimental.shard_map import shard_map
    from concourse import bass2jax

    nc = _get_nc()
    bass2jax.install_neuronx_cc_hook()
    partition_name = nc.partition_id_tensor.name if nc.partition_id_tensor else None
    in_names, out_names, out_avals, zero_outs = [], [], [], []
    for alloc in nc.m.functions[0].allocations:
        if not isinstance(alloc, mybir.MemoryLocationSet):
            continue
        name = alloc.memorylocations[0].name
        if alloc.kind == "ExternalInput":
            if name != partition_name:
                in_names.append(name)
        elif alloc.kind == "ExternalOutput":
            out_names.append(name)
            shape = tuple(alloc.tensor_shape)
            dtype = mybir.dt.np(alloc.dtype)
            out_avals.append(jax.core.ShapedArray(shape, dtype))
            zero_outs.append((shape, dtype))
    n_params = len(in_names)
    all_names = in_names + out_names
    if partition_name is not None:
        all_names = all_names + [partition_name]

    def _body(*args):
        operands = list(args)
        if partition_name is not None:
            operands.append(bass2jax.partition_id_tensor())
        return tuple(
            bass2jax._bass_exec_p.bind(
                *operands,
                out_avals=tuple(out_avals),
                in_names=tuple(all_names),
                out_names=tuple(out_names),
                lowering_input_output_aliases=(),
                sim_require_finite=True,
                sim_require_nnan=True,
                nc=nc,
            )
        )

    devices = jax.devices()[:N_CORES]
    mesh = Mesh(np.asarray(devices), ("core",))
    donate = tuple(range(n_params, n_params + len(out_names)))
    sharded = jax.jit(
        shard_map(
            _body,
            mesh=mesh,
            in_specs=(PartitionSpec("core"),) * (n_params + len(out_names)),
            out_specs=(PartitionSpec("core"),) * len(out_names),
            check_rep=False,
        ),
        donate_argnums=donate,
        keep_unused=True,
    )
    sh = NamedSharding(mesh, PartitionSpec("core"))
    _RUNNER = (sharded, sh, in_names, zero_outs, jax)
    return _RUNNER


def _kernel_fallback(arrs):
    nc = _get_nc()
    in_maps = [
        {n: a[c * BPC : (c + 1) * BPC] for n, a in arrs.items()}
        for c in range(N_CORES)
    ]
    res = run_bass_kernel_spmd(nc, in_maps, core_ids=list(range(N_CORES)))
    return np.concatenate([res.results[c]["out"] for c in range(N_CORES)], axis=0)


def kernel(q, k, v):
    arrs = {
        "q": np.ascontiguousarray(np.asarray(q), dtype=np.float32),
        "k": np.ascontiguousarray(np.asarray(k), dtype=np.float32),
        "v": np.ascontiguousarray(np.asarray(v), dtype=np.float32),
    }
    try:
        sharded, sh, in_names, zero_outs, jax = _get_runner()
        ins = [jax.device_put(arrs[n], sh) for n in in_names]
        zeros = [
            jax.device_put(np.zeros((N_CORES * s[0], *s[1:]), d), sh)
            for s, d in zero_outs
        ]
        out = sharded(*ins, *zeros)[0]
        return np.asarray(out).reshape(B, N, D)
    except Exception:
        return _kernel_fallback(arrs)
